# revision 2
# baseline (speedup 1.0000x reference)
"""Trainium2 Bass kernel v2: sparse-in -> dense-hidden -> sampled-out net.

  val1 = relu(in_values @ W1.T[active_in_indices] + b1)        # [B, H]
  val2 = einsum('bh,bkh->bk', val1, W2[active_label_indices]) + b2[...]

Strategy vs baseline: replace generic indirect-DMA row gathers (~70ns/descriptor
of serialized SWDGE descriptor generation) with the vectorized `dma_gather`
custom GPSIMD instruction, spread across up to 4 SWDGE queues (parallel Q7
descriptor generation). W2 is gathered in bf16 (256B rows, halves HBM traffic).

Layout trick: each core handles 16 samples; stage-2 gathered rows are arranged
on a fixed partition grid where partition p always belongs to sample p//8, so
the per-position "matching val1 row" tile V[p,:] = val1[p//8,:] is a constant
[128,128] tile and the dot is a DVE broadcast-multiply + free-dim reduce.
All index bucketing/sorting/padding and the final (grid -> [B,KOUT]) scatter
plus the b2 bias add happen on the host.
"""

import numpy as np
import ml_dtypes

B, NNZ, F_DIM, H, C, KOUT = 128, 128, 135909, 128, 670091, 4096
N_CORES = 8
BPC = B // N_CORES            # 16 samples per core
BK = 32768                    # index bucket size (int16-addressable)
NB1 = -(-F_DIM // BK)         # 5 buckets for W1 rows
NB2 = -(-C // BK)             # 21 buckets for W2 rows
W2_DTYPE = "bf16"             # gather W2 rows as bf16 (256B rows)
N_QUEUES = 4

_CACHE = {}


def build_program(slots1, slots2, n_queues=N_QUEUES, w2_dtype=W2_DTYPE):
    """slots1: stage-1 slots per bucket (each slot = 128 gathered rows).
    slots2: stage-2 slots per bucket (each slot = 128 rows, 8 per sample)."""
    import concourse.bass as bass
    import concourse.bacc as bacc
    import concourse.mybir as mybir
    import concourse.tile as tile

    fp32 = mybir.dt.float32
    bf16 = mybir.dt.bfloat16
    i16 = mybir.dt.int16
    w2dt = bf16 if w2_dtype == "bf16" else fp32

    T1 = NB1 * slots1
    T2 = NB2 * slots2
    n1 = 128 * slots1          # idx per stage-1 gather
    n2 = 128 * slots2          # idx per stage-2 gather

    nc = bacc.Bacc("TRN2", target_bir_lowering=False, debug=False,
                   num_swdge_queues=n_queues)

    w1t = nc.dram_tensor("w1t", [F_DIM, H], fp32, kind="ExternalInput")
    w2 = nc.dram_tensor("w2", [C, H], w2dt, kind="ExternalInput")
    idx1 = nc.dram_tensor("idx1", [128, NB1 * n1 // 16], i16, kind="ExternalInput")
    idx2 = nc.dram_tensor("idx2", [128, NB2 * n2 // 16], i16, kind="ExternalInput")
    s_w = nc.dram_tensor("s_w", [128, T1 * BPC], fp32, kind="ExternalInput")
    b1r = nc.dram_tensor("b1r", [BPC, H], fp32, kind="ExternalInput")
    e_t = nc.dram_tensor("e_t", [BPC, 128], bf16, kind="ExternalInput")
    out = nc.dram_tensor("val2g", [128, T2], fp32, kind="ExternalOutput")

    with tile.TileContext(nc) as tc:
        with (
            tc.tile_pool(name="const", bufs=1) as cpool,
            tc.tile_pool(name="g2", bufs=6) as g2pool,
            tc.tile_pool(name="prod", bufs=3) as prodpool,
            tc.tile_pool(name="psum", bufs=1, space="PSUM") as psum,
        ):
            idx1_t = cpool.tile([128, NB1 * n1 // 16], i16)
            nc.sync.dma_start(out=idx1_t[:], in_=idx1[:, :])
            idx2_t = cpool.tile([128, NB2 * n2 // 16], i16)
            nc.sync.dma_start(out=idx2_t[:], in_=idx2[:, :])
            s_t = cpool.tile([128, T1 * BPC], fp32)
            nc.sync.dma_start(out=s_t[:], in_=s_w[:, :])
            b1_t = cpool.tile([BPC, H], fp32)
            nc.sync.dma_start(out=b1_t[:], in_=b1r[:, :])
            e_tt = cpool.tile([BPC, 128], bf16)
            nc.sync.dma_start(out=e_tt[:], in_=e_t[:, :])

            # ---- stage-1 gathers (fp32 rows, 512B) ----
            q = 0
            g1_tiles = []
            CH1 = 4               # 4 slots = 512 idx per gather (fp32 rows)
            for k in range(NB1):
                blen = min(BK, F_DIM - k * BK)
                g1 = cpool.tile([128, slots1, H], fp32, tag=f"g1_{k}")
                for c0 in range(0, slots1, CH1):
                    c1 = min(c0 + CH1, slots1)
                    nidx = (c1 - c0) * 128
                    nc.gpsimd.dma_gather(
                        g1[:, c0:c1, :],
                        w1t[k * BK: k * BK + blen, :],
                        idx1_t[:, (k * n1 + c0 * 128) // 16:
                               (k * n1 + c1 * 128) // 16],
                        nidx, nidx, H,
                        queue_num=q,
                    )
                    q = (q + 1) % n_queues
                g1_tiles.append(g1)

            # ---- stage-2 gathers issued early (no dependence on stage 1) ----
            # single_packet concatenation is limited to 64 descriptors per
            # SDMA engine (= 1024 idx across 16 lanes); split bucket gathers.
            CH_SLOTS = 8          # 8 slots = 1024 idx per dma_gather
            g2_tiles = []
            for k in range(NB2):
                blen = min(BK, C - k * BK)
                g2 = g2pool.tile([128, slots2, H], w2dt, tag="g2")
                for c0 in range(0, slots2, CH_SLOTS):
                    c1 = min(c0 + CH_SLOTS, slots2)
                    nidx = (c1 - c0) * 128
                    nc.gpsimd.dma_gather(
                        g2[:, c0:c1, :],
                        w2[k * BK: k * BK + blen, :],
                        idx2_t[:, (k * n2 + c0 * 128) // 16:
                               (k * n2 + c1 * 128) // 16],
                        nidx, nidx, H,
                        queue_num=q,
                    )
                    q = (q + 1) % n_queues
                g2_tiles.append((k, g2))

            # ---- stage-1 matmuls: val1[b,h] = sum_pos S[pos,b] * G1[pos,h] ----
            v1_ps = psum.tile([BPC, H], fp32)
            mm = 0
            for k in range(NB1):
                for slot in range(slots1):
                    nc.tensor.matmul(
                        v1_ps[:],
                        lhsT=s_t[:, (k * slots1 + slot) * BPC:
                                 (k * slots1 + slot + 1) * BPC],
                        rhs=g1_tiles[k][:, slot, :],
                        start=(mm == 0),
                        stop=(mm == T1 - 1),
                    )
                    mm += 1

            # bias + relu -> bf16
            v1sum = cpool.tile([BPC, H], fp32)
            nc.vector.tensor_add(out=v1sum[:], in0=v1_ps[:], in1=b1_t[:])
            v1bf = cpool.tile([BPC, H], bf16)
            nc.vector.tensor_scalar_max(v1bf[:], v1sum[:], 0.0)

            # expand to V[p,:] = val1[p//8,:]
            v_ps = psum.tile([128, H], fp32)
            nc.tensor.matmul(v_ps[:], lhsT=e_tt[:], rhs=v1bf[:],
                             start=True, stop=True)
            v_t = cpool.tile([128, H], w2dt)
            nc.vector.tensor_copy(out=v_t[:], in_=v_ps[:])

            # ---- stage-2 dots: val2g[p, k*slots2+s] = <G2[p,s,:], V[p,:]> ----
            val2g = cpool.tile([128, T2], fp32)
            for k, g2 in g2_tiles:
                prod = prodpool.tile([128, slots2 * H], w2dt, tag="prod")
                nc.vector.tensor_tensor(
                    out=prod[:],
                    in0=g2[:],
                    in1=v_t[:]
                    .rearrange("p (o h) -> p o h", o=1)
                    .to_broadcast([128, slots2, H]),
                    op=mybir.AluOpType.mult,
                )
                nc.vector.tensor_reduce(
                    out=val2g[:, k * slots2: (k + 1) * slots2],
                    in_=prod[:].rearrange("p (s h) -> p s h", s=slots2),
                    axis=mybir.AxisListType.X,
                    op=mybir.AluOpType.add,
                )
            nc.sync.dma_start(out=out.ap(), in_=val2g[:])
    nc.finalize()
    return nc


def _wrap_idx(idx_arr):
    """[n] int -> [128, n//16] int16 wrapped in 16 partitions, replicated x8."""
    n = idx_arr.shape[0]
    w = idx_arr.reshape(n // 16, 16).T.astype(np.int16)
    return np.tile(w, (8, 1))


def make_core_inputs(in_values, active_in_indices, active_label_indices,
                     W1T, W2s, b1, w2_dtype=W2_DTYPE):
    """Host-side sharding, bucketing, grid layout. Returns (in_maps, slots1,
    slots2, scatter) where scatter[c] = (src_lin, dst_lin) for reassembly."""
    q1 = active_in_indices >> 15
    r1 = (active_in_indices & 32767).astype(np.int64)
    q2 = active_label_indices >> 15
    r2 = (active_label_indices & 32767).astype(np.int64)

    # shared capacities across cores
    cnt1 = np.zeros((N_CORES, NB1), np.int64)
    for c in range(N_CORES):
        cnt1[c] = np.bincount(q1[c * BPC:(c + 1) * BPC].ravel(), minlength=NB1)
    slots1 = int(-(-cnt1.max() // 128))

    cnt2 = np.zeros((B, NB2), np.int64)
    for b in range(B):
        cnt2[b] = np.bincount(q2[b], minlength=NB2)
    slots2 = int(-(-cnt2.max() // 8))

    T1 = NB1 * slots1
    T2 = NB2 * slots2
    n1 = 128 * slots1
    n2 = 128 * slots2

    b1r = np.ascontiguousarray(np.broadcast_to(b1.reshape(1, H), (BPC, H))
                               ).astype(np.float32)
    e_mat = np.zeros((BPC, 128), dtype=ml_dtypes.bfloat16)
    e_mat[np.arange(128) // 8, np.arange(128)] = 1.0

    in_maps = []
    scatter = []
    for c in range(N_CORES):
        s = slice(c * BPC, (c + 1) * BPC)
        ivc, q1c, r1c = in_values[s], q1[s], r1[s]
        q2c, r2c = q2[s], r2[s]

        # stage 1: per-bucket position lists + S matrix
        idx1_full = np.zeros(NB1 * n1, np.int64)
        S = np.zeros((128, T1, BPC), np.float32)
        for k in range(NB1):
            bs, iis = np.nonzero(q1c == k)
            m = np.arange(len(bs))
            slot, p = m // 128, m % 128
            idx1_full[k * n1 + m] = r1c[bs, iis]
            S[p, k * slots1 + slot, bs] = ivc[bs, iis]

        # stage 2: grid layout per bucket
        idx2_full = np.zeros(NB2 * n2, np.int64)
        src_parts = []
        dst_parts = []
        for k in range(NB2):
            for b in range(BPC):
                kpos = np.nonzero(q2c[b] == k)[0]
                m = np.arange(len(kpos))
                slot, col = m // 8, 8 * b + m % 8
                idx2_full[k * n2 + slot * 128 + col] = r2c[b][kpos]
                src_parts.append(col * T2 + k * slots2 + slot)
                dst_parts.append(b * KOUT + kpos)
        src_lin = np.concatenate(src_parts)
        dst_lin = np.concatenate(dst_parts)
        scatter.append((src_lin, dst_lin))

        in_maps.append({
            "w1t": W1T,
            "w2": W2s,
            "idx1": _wrap_idx(idx1_full),
            "idx2": _wrap_idx(idx2_full),
            "s_w": np.ascontiguousarray(S.reshape(128, T1 * BPC)),
            "b1r": b1r,
            "e_t": e_mat,
        })
    return in_maps, slots1, slots2, scatter


def kernel(in_values, active_in_indices, active_label_indices, W1, b1, W2, b2):
    from concourse.bass_utils import run_bass_kernel_spmd

    in_values = np.asarray(in_values, dtype=np.float32)
    active_in_indices = np.asarray(active_in_indices, dtype=np.int32)
    active_label_indices = np.asarray(active_label_indices, dtype=np.int32)
    W1 = np.asarray(W1, dtype=np.float32)
    b1 = np.asarray(b1, dtype=np.float32)
    W2 = np.asarray(W2, dtype=np.float32)
    b2 = np.asarray(b2, dtype=np.float32)

    W1T = np.ascontiguousarray(W1.T)
    W2s = W2.astype(ml_dtypes.bfloat16) if W2_DTYPE == "bf16" else W2
    in_maps, slots1, slots2, scatter = make_core_inputs(
        in_values, active_in_indices, active_label_indices, W1T, W2s, b1)

    key = (slots1, slots2)
    if _CACHE.get("key") != key:
        _CACHE["nc"] = build_program(slots1, slots2)
        _CACHE["key"] = key
    nc = _CACHE["nc"]

    res = run_bass_kernel_spmd(nc, in_maps, list(range(N_CORES)))
    T2 = NB2 * slots2
    val2 = np.empty((B, KOUT), dtype=np.float32)
    for c in range(N_CORES):
        src_lin, dst_lin = scatter[c]
        flat = np.asarray(res.results[c]["val2g"]).reshape(-1)
        val2[c * BPC:(c + 1) * BPC].reshape(-1)[dst_lin] = flat[src_lin]
    val2 += b2[active_label_indices]
    return val2, active_label_indices


# revision 3
# speedup vs baseline: 1.2000x; 1.2000x over previous
"""Trainium2 Bass kernel v2: sparse-in -> dense-hidden -> sampled-out net.

  val1 = relu(in_values @ W1.T[active_in_indices] + b1)        # [B, H]
  val2 = einsum('bh,bkh->bk', val1, W2[active_label_indices]) + b2[...]

Strategy vs baseline: replace generic indirect-DMA row gathers (~70ns/descriptor
of serialized SWDGE descriptor generation) with the vectorized `dma_gather`
custom GPSIMD instruction, spread across up to 4 SWDGE queues (parallel Q7
descriptor generation). W2 is gathered in bf16 (256B rows, halves HBM traffic).

Layout trick: each core handles 16 samples; stage-2 gathered rows are arranged
on a fixed partition grid where partition p always belongs to sample p//8, so
the per-position "matching val1 row" tile V[p,:] = val1[p//8,:] is a constant
[128,128] tile and the dot is a DVE broadcast-multiply + free-dim reduce.
All index bucketing/sorting/padding and the final (grid -> [B,KOUT]) scatter
plus the b2 bias add happen on the host.
"""

import numpy as np
import ml_dtypes

B, NNZ, F_DIM, H, C, KOUT = 128, 128, 135909, 128, 670091, 4096
N_CORES = 8
BPC = B // N_CORES            # 16 samples per core
BK = 32768                    # index bucket size (int16-addressable)
NB1 = -(-F_DIM // BK)         # 5 buckets for W1 rows
NB2 = -(-C // BK)             # 21 buckets for W2 rows
W2_DTYPE = "bf16"             # gather W2 rows as bf16 (256B rows)
N_QUEUES = 4

_CACHE = {}


def build_program(slots1, slots2, n_queues=N_QUEUES, w2_dtype=W2_DTYPE):
    """slots1: stage-1 slots per bucket (scalar; each slot = 128 rows).
    slots2: per-bucket stage-2 slot counts (list of NB2 ints)."""
    import concourse.bass as bass
    import concourse.bacc as bacc
    import concourse.mybir as mybir
    import concourse.tile as tile

    fp32 = mybir.dt.float32
    bf16 = mybir.dt.bfloat16
    i16 = mybir.dt.int16
    w2dt = bf16 if w2_dtype == "bf16" else fp32

    T1 = NB1 * slots1
    T2 = sum(slots2)
    n1 = 128 * slots1          # idx per stage-1 gather
    off2 = [0]
    for s in slots2:
        off2.append(off2[-1] + s)

    nc = bacc.Bacc("TRN2", target_bir_lowering=False, debug=False,
                   num_swdge_queues=n_queues)

    w1t = nc.dram_tensor("w1t", [F_DIM, H], fp32, kind="ExternalInput")
    w2 = nc.dram_tensor("w2", [C, H], w2dt, kind="ExternalInput")
    idx1 = nc.dram_tensor("idx1", [128, NB1 * n1 // 16], i16, kind="ExternalInput")
    idx2 = nc.dram_tensor("idx2", [128, T2 * 8], i16, kind="ExternalInput")
    s_w = nc.dram_tensor("s_w", [128, T1 * BPC], fp32, kind="ExternalInput")
    b1r = nc.dram_tensor("b1r", [BPC, H], fp32, kind="ExternalInput")
    e_t = nc.dram_tensor("e_t", [BPC, 128], bf16, kind="ExternalInput")
    out = nc.dram_tensor("val2g", [128, T2], fp32, kind="ExternalOutput")

    with tile.TileContext(nc) as tc:
        with (
            tc.tile_pool(name="const", bufs=1) as cpool,
            tc.tile_pool(name="g2", bufs=6) as g2pool,
            tc.tile_pool(name="prod", bufs=3) as prodpool,
            tc.tile_pool(name="psum", bufs=1, space="PSUM") as psum,
        ):
            idx1_t = cpool.tile([128, NB1 * n1 // 16], i16)
            nc.sync.dma_start(out=idx1_t[:], in_=idx1[:, :])
            idx2_t = cpool.tile([128, T2 * 8], i16)
            nc.sync.dma_start(out=idx2_t[:], in_=idx2[:, :])
            s_t = cpool.tile([128, T1 * BPC], fp32)
            nc.sync.dma_start(out=s_t[:], in_=s_w[:, :])
            b1_t = cpool.tile([BPC, H], fp32)
            nc.sync.dma_start(out=b1_t[:], in_=b1r[:, :])
            e_tt = cpool.tile([BPC, 128], bf16)
            nc.sync.dma_start(out=e_tt[:], in_=e_t[:, :])

            # ---- stage-1 gathers (fp32 rows, 512B) ----
            q = 0
            g1_tiles = []
            CH1 = 4               # 4 slots = 512 idx per gather (fp32 rows)
            for k in range(NB1):
                blen = min(BK, F_DIM - k * BK)
                g1 = cpool.tile([128, slots1, H], fp32, tag=f"g1_{k}")
                for c0 in range(0, slots1, CH1):
                    c1 = min(c0 + CH1, slots1)
                    nidx = (c1 - c0) * 128
                    nc.gpsimd.dma_gather(
                        g1[:, c0:c1, :],
                        w1t[k * BK: k * BK + blen, :],
                        idx1_t[:, (k * n1 + c0 * 128) // 16:
                               (k * n1 + c1 * 128) // 16],
                        nidx, nidx, H,
                        queue_num=q,
                    )
                    q = (q + 1) % n_queues
                g1_tiles.append(g1)

            # ---- stage-2 gathers issued early (no dependence on stage 1) ----
            # single_packet concatenation is limited to 64 descriptors per
            # SDMA engine (= 1024 idx across 16 lanes); split bucket gathers.
            CH_SLOTS = 8          # 8 slots = 1024 idx per dma_gather
            g2_tiles = []
            max_s2 = max(slots2)
            for k in range(NB2):
                blen = min(BK, C - k * BK)
                sk = slots2[k]
                g2 = g2pool.tile([128, max_s2, H], w2dt, tag="g2")
                for c0 in range(0, sk, CH_SLOTS):
                    c1 = min(c0 + CH_SLOTS, sk)
                    nidx = (c1 - c0) * 128
                    nc.gpsimd.dma_gather(
                        g2[:, c0:c1, :],
                        w2[k * BK: k * BK + blen, :],
                        idx2_t[:, (off2[k] + c0) * 8:
                               (off2[k] + c0) * 8 + nidx // 16],
                        nidx, nidx, H,
                        queue_num=q,
                    )
                    q = (q + 1) % n_queues
                g2_tiles.append((k, g2))

            # ---- stage-1 matmuls: val1[b,h] = sum_pos S[pos,b] * G1[pos,h] ----
            v1_ps = psum.tile([BPC, H], fp32)
            mm = 0
            for k in range(NB1):
                for slot in range(slots1):
                    nc.tensor.matmul(
                        v1_ps[:],
                        lhsT=s_t[:, (k * slots1 + slot) * BPC:
                                 (k * slots1 + slot + 1) * BPC],
                        rhs=g1_tiles[k][:, slot, :],
                        start=(mm == 0),
                        stop=(mm == T1 - 1),
                    )
                    mm += 1

            # bias + relu -> bf16
            v1sum = cpool.tile([BPC, H], fp32)
            nc.vector.tensor_add(out=v1sum[:], in0=v1_ps[:], in1=b1_t[:])
            v1bf = cpool.tile([BPC, H], bf16)
            nc.vector.tensor_scalar_max(v1bf[:], v1sum[:], 0.0)

            # expand to V[p,:] = val1[p//8,:]
            v_ps = psum.tile([128, H], fp32)
            nc.tensor.matmul(v_ps[:], lhsT=e_tt[:], rhs=v1bf[:],
                             start=True, stop=True)
            v_t = cpool.tile([128, H], w2dt)
            nc.vector.tensor_copy(out=v_t[:], in_=v_ps[:])

            # ---- stage-2 dots: val2g[p, k*slots2+s] = <G2[p,s,:], V[p,:]> ----
            val2g = cpool.tile([128, T2], fp32)
            for k, g2 in g2_tiles:
                sk = slots2[k]
                prod = prodpool.tile([128, max_s2 * H], w2dt, tag="prod")
                nc.vector.tensor_tensor(
                    out=prod[:, :sk * H],
                    in0=g2[:, :sk, :],
                    in1=v_t[:]
                    .rearrange("p (o h) -> p o h", o=1)
                    .to_broadcast([128, sk, H]),
                    op=mybir.AluOpType.mult,
                )
                nc.vector.tensor_reduce(
                    out=val2g[:, off2[k]: off2[k] + sk],
                    in_=prod[:, :sk * H].rearrange("p (s h) -> p s h", s=sk),
                    axis=mybir.AxisListType.X,
                    op=mybir.AluOpType.add,
                )
            nc.sync.dma_start(out=out.ap(), in_=val2g[:])
    nc.finalize()
    return nc


def _wrap_idx(idx_arr):
    """[n] int -> [128, n//16] int16 wrapped in 16 partitions, replicated x8."""
    n = idx_arr.shape[0]
    w = idx_arr.reshape(n // 16, 16).T.astype(np.int16)
    return np.tile(w, (8, 1))


def make_core_inputs(in_values, active_in_indices, active_label_indices,
                     W1T, W2s, b1, w2_dtype=W2_DTYPE):
    """Host-side sharding, bucketing, grid layout. Returns (in_maps, slots1,
    slots2, scatter) where scatter[c] = (src_lin, dst_lin) for reassembly."""
    q1 = active_in_indices >> 15
    r1 = (active_in_indices & 32767).astype(np.int64)
    q2 = active_label_indices >> 15
    r2 = (active_label_indices & 32767).astype(np.int64)

    # shared capacities across cores
    cnt1 = np.zeros((N_CORES, NB1), np.int64)
    for c in range(N_CORES):
        cnt1[c] = np.bincount(q1[c * BPC:(c + 1) * BPC].ravel(), minlength=NB1)
    slots1 = int(-(-cnt1.max() // 128))

    cnt2 = np.zeros((B, NB2), np.int64)
    for b in range(B):
        cnt2[b] = np.bincount(q2[b], minlength=NB2)
    # per-bucket slot counts: ceil(max over samples / 8), multiple of CH
    slots2 = [int(-(-int(cnt2[:, k].max()) // 8)) for k in range(NB2)]

    T1 = NB1 * slots1
    T2 = sum(slots2)
    n1 = 128 * slots1
    off2 = [0]
    for s in slots2:
        off2.append(off2[-1] + s)

    b1r = np.ascontiguousarray(np.broadcast_to(b1.reshape(1, H), (BPC, H))
                               ).astype(np.float32)
    e_mat = np.zeros((BPC, 128), dtype=ml_dtypes.bfloat16)
    e_mat[np.arange(128) // 8, np.arange(128)] = 1.0

    in_maps = []
    scatter = []
    for c in range(N_CORES):
        s = slice(c * BPC, (c + 1) * BPC)
        ivc, q1c, r1c = in_values[s], q1[s], r1[s]
        q2c, r2c = q2[s], r2[s]

        # stage 1: per-bucket position lists + S matrix
        idx1_full = np.zeros(NB1 * n1, np.int64)
        S = np.zeros((128, T1, BPC), np.float32)
        for k in range(NB1):
            bs, iis = np.nonzero(q1c == k)
            m = np.arange(len(bs))
            slot, p = m // 128, m % 128
            idx1_full[k * n1 + m] = r1c[bs, iis]
            S[p, k * slots1 + slot, bs] = ivc[bs, iis]

        # stage 2: grid layout per bucket
        idx2_full = np.zeros(T2 * 128, np.int64)
        src_parts = []
        dst_parts = []
        for k in range(NB2):
            for b in range(BPC):
                kpos = np.nonzero(q2c[b] == k)[0]
                m = np.arange(len(kpos))
                slot, col = m // 8, 8 * b + m % 8
                idx2_full[off2[k] * 128 + slot * 128 + col] = r2c[b][kpos]
                src_parts.append(col * T2 + off2[k] + slot)
                dst_parts.append(b * KOUT + kpos)
        src_lin = np.concatenate(src_parts)
        dst_lin = np.concatenate(dst_parts)
        scatter.append((src_lin, dst_lin))

        in_maps.append({
            "w1t": W1T,
            "w2": W2s,
            "idx1": _wrap_idx(idx1_full),
            "idx2": _wrap_idx(idx2_full),
            "s_w": np.ascontiguousarray(S.reshape(128, T1 * BPC)),
            "b1r": b1r,
            "e_t": e_mat,
        })
    return in_maps, slots1, slots2, scatter


def kernel(in_values, active_in_indices, active_label_indices, W1, b1, W2, b2):
    from concourse.bass_utils import run_bass_kernel_spmd

    in_values = np.asarray(in_values, dtype=np.float32)
    active_in_indices = np.asarray(active_in_indices, dtype=np.int32)
    active_label_indices = np.asarray(active_label_indices, dtype=np.int32)
    W1 = np.asarray(W1, dtype=np.float32)
    b1 = np.asarray(b1, dtype=np.float32)
    W2 = np.asarray(W2, dtype=np.float32)
    b2 = np.asarray(b2, dtype=np.float32)

    W1T = np.ascontiguousarray(W1.T)
    W2s = W2.astype(ml_dtypes.bfloat16) if W2_DTYPE == "bf16" else W2
    in_maps, slots1, slots2, scatter = make_core_inputs(
        in_values, active_in_indices, active_label_indices, W1T, W2s, b1)

    key = (slots1, tuple(slots2))
    if _CACHE.get("key") != key:
        _CACHE["nc"] = build_program(slots1, slots2)
        _CACHE["key"] = key
    nc = _CACHE["nc"]

    res = run_bass_kernel_spmd(nc, in_maps, list(range(N_CORES)))
    T2 = sum(slots2)
    val2 = np.empty((B, KOUT), dtype=np.float32)
    for c in range(N_CORES):
        src_lin, dst_lin = scatter[c]
        flat = np.asarray(res.results[c]["val2g"]).reshape(-1)
        val2[c * BPC:(c + 1) * BPC].reshape(-1)[dst_lin] = flat[src_lin]
    val2 += b2[active_label_indices]
    return val2, active_label_indices
